# revision 25
# baseline (speedup 1.0000x reference)
"""KANvolution Trainium2 Bass kernel (v4: hat basis, bf16, col-tiled PE,
PE-side input replication, two-phase chunk pipeline, fp16 output).

Math: per patch element x and per (f,c,ki,kj):
    K(x) = w_spline * sum_g hat_g(clip(x)) * cp_g  +  w_silu * silu(x)
with hat_g the normalized linear B-spline (tent) basis on the 17-knot
grid in [-1,1] (hat sum == 1, so the reference's /(sum+1e-8) is a
constant 1/(1+1e-8) scale folded into the weights).

The tent basis is evaluated directly:
    nhat_g(x) = min(|8x - k_g| - 1, 0)      ( = -hat_g,  k_g = 8*grid_g )
with spline weights negated host-side.  Clipping x to [-1,1] only
matters for the two edge knots (interior tents vanish outside the grid
anyway), so knots 1..15 are computed straight from x and the edge knots
live in the off-critical-path tail k-tile.

k-tiles per tap (ki,kj):
    t=0..3 : knots 4t+1 .. 4t+4  x 32 ch   (slot (3,3) is a zero-weight pad)
    t=4    : hat0(32) + hat16(32) + silu(32) + bias-ones(1) = 97 rows
9 taps x 5 k-tiles x 4 row-chunks = 180 matmuls [K<=128, N=64] x [K, 512].

PE usage: F=64 fills half the 128-wide array, so matmuls alternate
between array column groups (tile_position via PSUM slice base
partition); the groups stream concurrently through separate XBUSes.
Each (t,tap) weight is loaded once per chunk-pair (LDWEIGHTS hides
under the other group's matmuls).  Chunks {0,1} complete in phase 1 so
their output DMA overlaps phase 2's matmuls.

Input x is DMAed once ([32, 2304] bf16) and replicated to 128
partitions by a K=32 matmul against a replication matrix (this also
warms the PE clock).  Outputs leave as fp16 per-column-group halves;
the host sums them.

Sharding: 8 cores = (batch b, output-row half); each core computes
(32, 64, 64) of the output.
"""

import numpy as np
from contextlib import ExitStack

import concourse.bacc as bacc
import concourse.mybir as mybir
import concourse.tile as tile
from concourse.bass_utils import run_bass_kernel_spmd

# Problem constants (hardcoded per harness contract)
B, H, W, C, F = 4, 66, 66, 32, 64
KH = KW = 3
G = 16                                   # spline intervals; G+1 = 17 knots
HO, WO = H - KH + 1, W - KW + 1          # 64, 64
N_CORES = 8
ROWS_PER_CORE = HO // 2                  # 32 output rows
IN_ROWS = ROWS_PER_CORE + KH - 1         # 34 input rows
SPAT = IN_ROWS * W                       # 2244 input spatial positions
SPAT_PAD = 2304                          # pad to 18*128
N_TAPS = KH * KW                         # 9
N_KTILES = 5
K_TAIL = 97                              # hat0(32)+hat16(32)+silu(32)+bias(1)
CHUNK_ROWS = 8                           # output rows per matmul chunk
N_CHUNKS = ROWS_PER_CORE // CHUNK_ROWS   # 4
NFREE = CHUNK_ROWS * WO                  # 512 moving-dim per matmul
SLABS = [(0, 576), (576, 1152), (1152, 1728), (1728, 2304)]  # feature slabs
N_WARMUP = 8                             # HAM warm-up matmuls (K=128)

_COMPILED = None  # cached (nc) program


def _build_weights(control_points, w_spline, w_silu, bias):
    """[128, 45*64] bf16; column block (t*9+tap)*64 holds k-tile t of tap.

    t<4 : row r*32+c = knot 4t+r+1 (negated); slot (t,r)==(3,3) is zero.
    t=4 : rows 0-31 knot 0 (pos), 32-63 knot 16 (pos), 64-95 w_silu,
          row 96 bias (tap 0 only).
    """
    import ml_dtypes
    cp = control_points.astype(np.float64)
    ws = w_spline.astype(np.float64)
    v = ws[..., None] * cp / (1.0 + 1e-8)          # (F, C, 3, 3, 17)

    w_all = np.zeros((N_KTILES, N_TAPS, 128, F), dtype=np.float64)
    for i in range(KH):
        for j in range(KW):
            tap = i * KW + j
            for t in range(4):
                for r in range(4):
                    if t == 3 and r == 3:
                        continue                   # zero-weight pad slot
                    g = 4 * t + r + 1
                    w_all[t, tap, r * 32:(r + 1) * 32, :] = -v[:, :, i, j, g].T
            w_all[4, tap, 0:32, :] = v[:, :, i, j, 0].T
            w_all[4, tap, 32:64, :] = v[:, :, i, j, 16].T
            w_all[4, tap, 64:96, :] = w_silu[:, :, i, j].astype(np.float64).T
    w_all[4, 0, 96, :] = bias.astype(np.float64)
    w_host = w_all.transpose(2, 0, 1, 3).reshape(128, N_KTILES * N_TAPS * F)
    return np.ascontiguousarray(w_host.astype(ml_dtypes.bfloat16))


def _build_program():
    nc = bacc.Bacc("TRN2", target_bir_lowering=False, debug=False,
                   num_devices=N_CORES)
    f32 = mybir.dt.float32
    bf16 = mybir.dt.bfloat16
    fp16 = mybir.dt.float16
    AF = mybir.ActivationFunctionType
    OP = mybir.AluOpType
    import os
    # CoreSim has no Silu; swap in Sigmoid for sim-only structure checks.
    AF_SILU = AF.Sigmoid if os.environ.get("KAN_SIM_SAFE") else AF.Silu

    x_in = nc.declare_dram_parameter("x8t", [32, SPAT_PAD + 128], bf16,
                                    isOutput=False)
    w_in = nc.declare_dram_parameter("w", [128, N_KTILES * N_TAPS * F], bf16,
                                     isOutput=False)
    kv_in = nc.declare_dram_parameter("kv", [128, 8], f32, isOutput=False)
    ones_in = nc.declare_dram_parameter("ones", [1, SPAT_PAD], bf16,
                                        isOutput=False)
    y_out = nc.declare_dram_parameter("y", [128, N_CHUNKS * NFREE], fp16,
                                      isOutput=True)

    with tile.TileContext(nc) as tc:
        with ExitStack() as ctx:
            sb = ctx.enter_context(tc.tile_pool(name="sb", bufs=1))
            ps = ctx.enter_context(tc.tile_pool(name="ps", bufs=1, space="PSUM"))
            ob = ctx.enter_context(tc.tile_pool(name="ob", bufs=1))

            # --- input DMAs: sync and gpsimd queues, need-order; the
            # replication matrix rides in x8t's last 128 columns ---
            kv_sb = sb.tile([128, 8], f32, tag="kv")
            nc.sync.dma_start(kv_sb[:], kv_in[:])
            x_sb = sb.tile([32, SPAT_PAD + 128], bf16, tag="xsb")
            nc.sync.dma_start(x_sb[0:16, :], x_in[0:16, :])
            nc.scalar.dma_start(x_sb[16:32, :], x_in[16:32, :])
            rep_sb = x_sb[:, SPAT_PAD:SPAT_PAD + 128]
            w_sb = sb.tile([128, N_KTILES * N_TAPS * F], bf16, tag="w")
            nc.gpsimd.dma_start(w_sb[:, 0:576], w_in[:, 0:576])        # t=0
            nc.sync.dma_start(w_sb[:, 576:1152], w_in[:, 576:1152])    # t=1
            nc.gpsimd.dma_start(w_sb[:, 1152:1728], w_in[:, 1152:1728])
            nc.sync.dma_start(w_sb[:, 1728:2304], w_in[:, 1728:2304])
            nc.gpsimd.dma_start(w_sb[:, 2304:2880], w_in[:, 2304:2880])

            # feature tiles
            x_rep = sb.tile([128, SPAT_PAD], bf16, tag="xrep")
            u64 = sb.tile([64, SPAT_PAD], bf16, tag="u64")
            tb = [sb.tile([128, SPAT_PAD], bf16, name=f"tb{u}", tag=f"tb{u}")
                  for u in range(2)]
            nhat = [sb.tile([128, SPAT_PAD], bf16, name=f"nh{t}", tag=f"nh{t}")
                    for t in range(N_KTILES)]
            nc.sync.dma_start(nhat[4][96:97, :], ones_in[:])   # bias row = 1.0

            # PSUM: 4 chunk tiles x 2 banks (col-group A in bank 0 /
            # partitions 0-63, group B in bank 1 / partitions 64-127 — one
            # accumulation group per zero region).
            P = [ps.tile([128, 2 * NFREE], f32, name=f"po{q}", tag=f"po{q}")
                 for q in range(N_CHUNKS)]

            # HAM warm-up: K=128 junk matmuls on the t0 weight block (K=32
            # matmuls don't generate enough array activity to unthrottle the
            # clock).  They write P[3]'s B region; the real accumulation's
            # start=True clears it.
            for u in range(N_WARMUP):
                nc.tensor.matmul(P[3][64:128, NFREE:2 * NFREE],
                                 w_sb[:, 0:F], w_sb[:, 0:NFREE],
                                 start=True, stop=True)

            # --- replicate x to 128 partitions via K=32 matmuls against
            # the replication matrix ---
            for u in range(5):
                c0 = 512 * u
                c1 = min(c0 + 512, SPAT_PAD)
                pr = P[u % 4][:, (u // 4) * NFREE:(u // 4) * NFREE + c1 - c0]
                nc.tensor.matmul(pr, rep_sb, x_sb[:, c0:c1],
                                 start=True, stop=True)
                nc.vector.tensor_copy(x_rep[:, c0:c1], pr)

            def features(t):
                """All three column slabs of k-tile t (t-major emission)."""
                for a, b in SLABS:
                    cs = slice(a, b)
                    if t < 4:
                        tbt = tb[t % 2]
                        # |x8 - k| on ACT (per-partition bias = -k); interior
                        # knots need no clipping.
                        nc.scalar.activation(tbt[:, cs], x_rep[:, cs], AF.Abs,
                                             bias=kv_sb[:, t:t + 1], scale=1.0)
                        nc.vector.tensor_scalar(nhat[t][:, cs], tbt[:, cs],
                                                1.0, 0.0, OP.subtract, OP.min)
                    else:
                        # tail: hat0 = clamp(-x8-7, 0, 1) (rows 0-31),
                        # hat16 = clamp(x8-7, 0, 1) (rows 32-63) — positive
                        # tents, weights not negated; silu from raw x.
                        nc.vector.tensor_scalar(u64[:, cs], x_rep[0:64, cs],
                                                kv_sb[0:64, 4:5], -7.0,
                                                OP.mult, OP.add)
                        nc.vector.tensor_scalar(nhat[4][0:64, cs], u64[:, cs],
                                                0.0, 1.0, OP.max, OP.min)
                        nc.scalar.activation(nhat[4][64:96, cs],
                                             x_rep[32:64, cs],
                                             AF_SILU, scale=0.125)

            order = [(t, tap) for t in range(N_KTILES) for tap in range(N_TAPS)]
            started = set()

            def emit_mm(n, t, tap, qlist, stop):
                g_ = n % 2
                i, j = divmod(tap, KW)
                kk = 128 if t < 4 else K_TAIL
                col = (t * N_TAPS + tap) * F
                lhsT = w_sb[0:kk, col:col + F]
                for q in qlist:
                    base = (CHUNK_ROWS * q + i) * W
                    rhs = (nhat[t][0:kk, base:base + CHUNK_ROWS * W]
                           .rearrange("p (r w) -> p r w", w=W)
                           [:, :, j:j + WO])
                    nc.tensor.matmul(
                        P[q][F * g_:F * (g_ + 1),
                             NFREE * g_:NFREE * (g_ + 1)]
                            .rearrange("f (r w) -> f r w", w=WO),
                        lhsT, rhs,
                        start=((q, g_) not in started), stop=stop,
                    )
                    started.add((q, g_))

            def emit_out(q):
                # PSUM -> SBUF as fp16 halves (host sums them); each half
                # DMAs from its own queue as soon as its copy lands.
                stage = ob.tile([128, NFREE], fp16, tag=f"stage{q}")
                nc.scalar.copy(stage[0:F, :], P[q][0:F, 0:NFREE])
                nc.sync.dma_start(y_out[0:F, NFREE * q:NFREE * (q + 1)],
                                  stage[0:F, :])
                nc.vector.tensor_copy(stage[F:128, :],
                                      P[q][F:128, NFREE:2 * NFREE])
                nc.gpsimd.dma_start(y_out[F:128, NFREE * q:NFREE * (q + 1)],
                                    stage[F:128, :])

            # hat tiles t0-t3 for all 4 chunks (weights stay loaded across
            # the 4 matmuls); the tail tile closes chunks {0,1} first so
            # their output DMA overlaps the rest.
            for n, (t, tap) in enumerate(order):
                if tap == 0:
                    features(t)
                if t < 4:
                    emit_mm(n, t, tap, (0, 1, 2, 3), stop=False)
            for n, (t, tap) in enumerate(order):
                if t == 4:
                    emit_mm(n, t, tap, (0, 1), stop=(n >= len(order) - 2))
            emit_out(0)
            emit_out(1)
            for n, (t, tap) in enumerate(order):
                if t == 4:
                    emit_mm(n, t, tap, (2, 3), stop=(n >= len(order) - 2))
            emit_out(2)
            emit_out(3)

    nc.compile()
    return nc


def _get_program():
    global _COMPILED
    if _COMPILED is None:
        _COMPILED = _build_program()
    return _COMPILED


def _make_in_maps(x, control_points, w_spline, w_silu, bias):
    import ml_dtypes
    bf = ml_dtypes.bfloat16
    w_host = _build_weights(control_points, w_spline, w_silu, bias)

    # ACT Abs bias constants: tb = Abs(x8 + kv); interior knot index
    # n = 4t + r + 1 sits at k = n - 8, so kv = 8 - n.
    kv = np.zeros((128, 8), dtype=np.float32)
    for t in range(4):
        for p in range(128):
            kv[p, t] = 8.0 - (4 * t + p // 32 + 1)
    kv[0:32, 4] = -1.0     # tail rows 0-31: hat0 ramp = -x8 - 7
    kv[32:64, 4] = 1.0     # tail rows 32-63: hat16 ramp = x8 - 7

    rep = np.zeros((32, 128), dtype=bf)
    for p in range(128):
        rep[p % 32, p] = 1.0
    ones = np.ones((1, SPAT_PAD), dtype=bf)

    x8 = (np.asarray(x, dtype=np.float32) * 8.0).astype(bf)
    in_maps = []
    for core in range(N_CORES):
        b, half = divmod(core, 2)
        r0 = half * ROWS_PER_CORE
        xs = np.zeros((32, SPAT_PAD + 128), dtype=bf)
        xs[:, :SPAT] = x8[b, r0:r0 + IN_ROWS].reshape(SPAT, C).T
        xs[:, SPAT_PAD:] = rep
        in_maps.append({"x8t": xs, "w": w_host, "kv": kv, "ones": ones})
    return in_maps


def kernel(x, control_points, w_spline, w_silu, bias):
    in_maps = _make_in_maps(x, control_points, w_spline, w_silu, bias)
    nc = _get_program()
    res = run_bass_kernel_spmd(nc, in_maps, list(range(N_CORES)))

    out = np.empty((B, HO, WO, F), dtype=np.float32)
    for core in range(N_CORES):
        b, half = divmod(core, 2)
        r0 = half * ROWS_PER_CORE
        y2 = res.results[core]["y"].astype(np.float32)   # [128, 2048] fp16
        y = y2[0:F] + y2[F:128]                          # [64, 2048]
        out[b, r0:r0 + ROWS_PER_CORE] = (
            y.reshape(F, ROWS_PER_CORE, WO).transpose(1, 2, 0))
    return out


# revision 26
# speedup vs baseline: 1.0475x; 1.0475x over previous
"""KANvolution Trainium2 Bass kernel (v4: hat basis, bf16, col-tiled PE,
PE-side input replication, two-phase chunk pipeline, fp16 output).

Math: per patch element x and per (f,c,ki,kj):
    K(x) = w_spline * sum_g hat_g(clip(x)) * cp_g  +  w_silu * silu(x)
with hat_g the normalized linear B-spline (tent) basis on the 17-knot
grid in [-1,1] (hat sum == 1, so the reference's /(sum+1e-8) is a
constant 1/(1+1e-8) scale folded into the weights).

The tent basis is evaluated directly:
    nhat_g(x) = min(|8x - k_g| - 1, 0)      ( = -hat_g,  k_g = 8*grid_g )
with spline weights negated host-side.  Clipping x to [-1,1] only
matters for the two edge knots (interior tents vanish outside the grid
anyway), so knots 1..15 are computed straight from x and the edge knots
live in the off-critical-path tail k-tile.

k-tiles per tap (ki,kj):
    t=0..3 : knots 4t+1 .. 4t+4  x 32 ch   (slot (3,3) is a zero-weight pad)
    t=4    : hat0(32) + hat16(32) + silu(32) + bias-ones(1) = 97 rows
9 taps x 5 k-tiles x 4 row-chunks = 180 matmuls [K<=128, N=64] x [K, 512].

PE usage: F=64 fills half the 128-wide array, so matmuls alternate
between array column groups (tile_position via PSUM slice base
partition); the groups stream concurrently through separate XBUSes.
Each (t,tap) weight is loaded once per chunk-pair (LDWEIGHTS hides
under the other group's matmuls).  Chunks {0,1} complete in phase 1 so
their output DMA overlaps phase 2's matmuls.

Input x is DMAed once ([32, 2304] bf16) and replicated to 128
partitions by a K=32 matmul against a replication matrix (this also
warms the PE clock).  Outputs leave as fp16 per-column-group halves;
the host sums them.

Sharding: 8 cores = (batch b, output-row half); each core computes
(32, 64, 64) of the output.
"""

import numpy as np
from contextlib import ExitStack

import concourse.bacc as bacc
import concourse.mybir as mybir
import concourse.tile as tile
from concourse.bass_utils import run_bass_kernel_spmd

# Problem constants (hardcoded per harness contract)
B, H, W, C, F = 4, 66, 66, 32, 64
KH = KW = 3
G = 16                                   # spline intervals; G+1 = 17 knots
HO, WO = H - KH + 1, W - KW + 1          # 64, 64
N_CORES = 8
ROWS_PER_CORE = HO // 2                  # 32 output rows
IN_ROWS = ROWS_PER_CORE + KH - 1         # 34 input rows
SPAT = IN_ROWS * W                       # 2244 input spatial positions
SPAT_PAD = 2304                          # pad to 18*128
N_TAPS = KH * KW                         # 9
N_KTILES = 5
K_TAIL = 97                              # hat0(32)+hat16(32)+silu(32)+bias(1)
CHUNK_ROWS = 8                           # output rows per matmul chunk
N_CHUNKS = ROWS_PER_CORE // CHUNK_ROWS   # 4
NFREE = CHUNK_ROWS * WO                  # 512 moving-dim per matmul
SLABS = [(0, 576), (576, 1152), (1152, 1728), (1728, 2304)]  # feature slabs
N_WARMUP = 8                             # HAM warm-up matmuls (K=128)

_COMPILED = None  # cached (nc) program


def _build_weights(control_points, w_spline, w_silu, bias):
    """[128, 45*64] bf16; column block (t*9+tap)*64 holds k-tile t of tap.

    t<4 : row r*32+c = knot 4t+r+1 (negated); slot (t,r)==(3,3) is zero.
    t=4 : rows 0-31 knot 0 (pos), 32-63 knot 16 (pos), 64-95 w_silu,
          row 96 bias (tap 0 only).
    """
    import ml_dtypes
    cp = control_points.astype(np.float64)
    ws = w_spline.astype(np.float64)
    v = ws[..., None] * cp / (1.0 + 1e-8)          # (F, C, 3, 3, 17)

    w_all = np.zeros((N_KTILES, N_TAPS, 128, F), dtype=np.float64)
    for i in range(KH):
        for j in range(KW):
            tap = i * KW + j
            for t in range(4):
                for r in range(4):
                    if t == 3 and r == 3:
                        continue                   # zero-weight pad slot
                    g = 4 * t + r + 1
                    w_all[t, tap, r * 32:(r + 1) * 32, :] = -v[:, :, i, j, g].T
            w_all[4, tap, 0:32, :] = v[:, :, i, j, 0].T
            w_all[4, tap, 32:64, :] = v[:, :, i, j, 16].T
            w_all[4, tap, 64:96, :] = w_silu[:, :, i, j].astype(np.float64).T
    w_all[4, 0, 96, :] = bias.astype(np.float64)
    w_host = w_all.transpose(2, 0, 1, 3).reshape(128, N_KTILES * N_TAPS * F)
    return np.ascontiguousarray(w_host.astype(ml_dtypes.bfloat16))


def _build_program():
    nc = bacc.Bacc("TRN2", target_bir_lowering=False, debug=False,
                   num_devices=N_CORES)
    f32 = mybir.dt.float32
    bf16 = mybir.dt.bfloat16
    fp16 = mybir.dt.float16
    AF = mybir.ActivationFunctionType
    OP = mybir.AluOpType
    import os
    # CoreSim has no Silu; swap in Sigmoid for sim-only structure checks.
    AF_SILU = AF.Sigmoid if os.environ.get("KAN_SIM_SAFE") else AF.Silu

    x_in = nc.declare_dram_parameter("x8t", [32, SPAT_PAD + 128], bf16,
                                    isOutput=False)
    w_in = nc.declare_dram_parameter("w", [128, N_KTILES * N_TAPS * F], bf16,
                                     isOutput=False)
    kv_in = nc.declare_dram_parameter("kv", [128, 8], f32, isOutput=False)
    ones_in = nc.declare_dram_parameter("ones", [1, SPAT_PAD], bf16,
                                        isOutput=False)
    y_out = nc.declare_dram_parameter("y", [128, N_CHUNKS * NFREE], fp16,
                                      isOutput=True)

    with tile.TileContext(nc) as tc:
        with ExitStack() as ctx:
            sb = ctx.enter_context(tc.tile_pool(name="sb", bufs=1))
            ps = ctx.enter_context(tc.tile_pool(name="ps", bufs=1, space="PSUM"))
            ob = ctx.enter_context(tc.tile_pool(name="ob", bufs=1))

            # --- input DMAs: all three DMA-capable queues, need-order;
            # the replication matrix rides in x8t's last 128 columns ---
            kv_sb = sb.tile([128, 8], f32, tag="kv")
            nc.sync.dma_start(kv_sb[:], kv_in[:])
            x_sb = sb.tile([32, SPAT_PAD + 128], bf16, tag="xsb")
            nc.sync.dma_start(x_sb[0:11, :], x_in[0:11, :])
            nc.scalar.dma_start(x_sb[11:22, :], x_in[11:22, :])
            nc.gpsimd.dma_start(x_sb[22:32, :], x_in[22:32, :])
            rep_sb = x_sb[:, SPAT_PAD:SPAT_PAD + 128]
            w_sb = sb.tile([128, N_KTILES * N_TAPS * F], bf16, tag="w")
            nc.sync.dma_start(w_sb[0:44, 0:576], w_in[0:44, 0:576])    # t=0
            nc.scalar.dma_start(w_sb[44:88, 0:576], w_in[44:88, 0:576])
            nc.gpsimd.dma_start(w_sb[88:128, 0:576], w_in[88:128, 0:576])
            nc.gpsimd.dma_start(w_sb[:, 576:1152], w_in[:, 576:1152])  # t=1
            nc.sync.dma_start(w_sb[:, 1152:1728], w_in[:, 1152:1728])
            nc.scalar.dma_start(w_sb[:, 1728:2304], w_in[:, 1728:2304])
            nc.gpsimd.dma_start(w_sb[:, 2304:2880], w_in[:, 2304:2880])

            # feature tiles
            x_rep = sb.tile([128, SPAT_PAD], bf16, tag="xrep")
            u64 = sb.tile([64, SPAT_PAD], bf16, tag="u64")
            tb = [sb.tile([128, SPAT_PAD], bf16, name=f"tb{u}", tag=f"tb{u}")
                  for u in range(2)]
            nhat = [sb.tile([128, SPAT_PAD], bf16, name=f"nh{t}", tag=f"nh{t}")
                    for t in range(N_KTILES)]
            nc.sync.dma_start(nhat[4][96:97, :], ones_in[:])   # bias row = 1.0

            # PSUM: 4 chunk tiles x 2 banks (col-group A in bank 0 /
            # partitions 0-63, group B in bank 1 / partitions 64-127 — one
            # accumulation group per zero region).
            P = [ps.tile([128, 2 * NFREE], f32, name=f"po{q}", tag=f"po{q}")
                 for q in range(N_CHUNKS)]

            # HAM warm-up: K=128 junk matmuls on a zero-filled tile (K=32
            # matmuls don't generate enough array activity to unthrottle
            # the clock).  They write P[3]'s B region; the real
            # accumulation's start=True clears it.  Phase A runs before the
            # input-dependent work, phase B bridges until features land.
            nc.vector.memset(tb[0][:, 0:NFREE], 0.0)
            for u in range(N_WARMUP):
                nc.tensor.matmul(P[3][64:128, NFREE:2 * NFREE],
                                 tb[0][:, 0:F], tb[0][:, 0:NFREE],
                                 start=True, stop=True)

            # --- replicate x to 128 partitions via K=32 matmuls against
            # the replication matrix ---
            for u in range(5):
                c0 = 512 * u
                c1 = min(c0 + 512, SPAT_PAD)
                pr = P[u % 4][:, (u // 4) * NFREE:(u // 4) * NFREE + c1 - c0]
                nc.tensor.matmul(pr, rep_sb, x_sb[:, c0:c1],
                                 start=True, stop=True)
                nc.vector.tensor_copy(x_rep[:, c0:c1], pr)
            for u in range(4):      # phase-B warm-up bridge
                nc.tensor.matmul(P[3][64:128, NFREE:2 * NFREE],
                                 tb[0][:, 0:F], tb[0][:, 0:NFREE],
                                 start=True, stop=True)

            def features(t):
                """All three column slabs of k-tile t (t-major emission)."""
                for a, b in SLABS:
                    cs = slice(a, b)
                    if t < 4:
                        tbt = tb[t % 2]
                        # |x8 - k| on ACT (per-partition bias = -k); interior
                        # knots need no clipping.
                        nc.scalar.activation(tbt[:, cs], x_rep[:, cs], AF.Abs,
                                             bias=kv_sb[:, t:t + 1], scale=1.0)
                        nc.vector.tensor_scalar(nhat[t][:, cs], tbt[:, cs],
                                                1.0, 0.0, OP.subtract, OP.min)
                    else:
                        # tail: hat0 = clamp(-x8-7, 0, 1) (rows 0-31),
                        # hat16 = clamp(x8-7, 0, 1) (rows 32-63) — positive
                        # tents, weights not negated; silu from raw x.
                        nc.vector.tensor_scalar(u64[:, cs], x_rep[0:64, cs],
                                                kv_sb[0:64, 4:5], -7.0,
                                                OP.mult, OP.add)
                        nc.vector.tensor_scalar(nhat[4][0:64, cs], u64[:, cs],
                                                0.0, 1.0, OP.max, OP.min)
                        nc.scalar.activation(nhat[4][64:96, cs],
                                             x_rep[32:64, cs],
                                             AF_SILU, scale=0.125)

            order = [(t, tap) for t in range(N_KTILES) for tap in range(N_TAPS)]
            started = set()

            def emit_mm(n, t, tap, qlist, stop):
                g_ = n % 2
                i, j = divmod(tap, KW)
                kk = 128 if t < 4 else K_TAIL
                col = (t * N_TAPS + tap) * F
                lhsT = w_sb[0:kk, col:col + F]
                for q in qlist:
                    base = (CHUNK_ROWS * q + i) * W
                    rhs = (nhat[t][0:kk, base:base + CHUNK_ROWS * W]
                           .rearrange("p (r w) -> p r w", w=W)
                           [:, :, j:j + WO])
                    nc.tensor.matmul(
                        P[q][F * g_:F * (g_ + 1),
                             NFREE * g_:NFREE * (g_ + 1)]
                            .rearrange("f (r w) -> f r w", w=WO),
                        lhsT, rhs,
                        start=((q, g_) not in started), stop=stop,
                    )
                    started.add((q, g_))

            def emit_out(q):
                # PSUM -> SBUF as fp16 halves (host sums them); each half
                # DMAs from its own queue as soon as its copy lands.
                stage = ob.tile([128, NFREE], fp16, tag=f"stage{q}")
                nc.scalar.copy(stage[0:F, :], P[q][0:F, 0:NFREE])
                nc.sync.dma_start(y_out[0:F, NFREE * q:NFREE * (q + 1)],
                                  stage[0:F, :])
                nc.vector.tensor_copy(stage[F:128, :],
                                      P[q][F:128, NFREE:2 * NFREE])
                nc.gpsimd.dma_start(y_out[F:128, NFREE * q:NFREE * (q + 1)],
                                    stage[F:128, :])

            # hat tiles t0-t3 for all 4 chunks (weights stay loaded across
            # the 4 matmuls); the tail tile closes chunks {0,1} first so
            # their output DMA overlaps the rest.
            for n, (t, tap) in enumerate(order):
                if tap == 0:
                    features(t)
                if t < 4:
                    emit_mm(n, t, tap, (0, 1, 2, 3), stop=False)
            for n, (t, tap) in enumerate(order):
                if t == 4:
                    emit_mm(n, t, tap, (0, 1), stop=(n >= len(order) - 2))
            emit_out(0)
            emit_out(1)
            for n, (t, tap) in enumerate(order):
                if t == 4:
                    emit_mm(n, t, tap, (2, 3), stop=(n >= len(order) - 2))
            emit_out(2)
            emit_out(3)

    nc.compile()
    return nc


def _get_program():
    global _COMPILED
    if _COMPILED is None:
        _COMPILED = _build_program()
    return _COMPILED


def _make_in_maps(x, control_points, w_spline, w_silu, bias):
    import ml_dtypes
    bf = ml_dtypes.bfloat16
    w_host = _build_weights(control_points, w_spline, w_silu, bias)

    # ACT Abs bias constants: tb = Abs(x8 + kv); interior knot index
    # n = 4t + r + 1 sits at k = n - 8, so kv = 8 - n.
    kv = np.zeros((128, 8), dtype=np.float32)
    for t in range(4):
        for p in range(128):
            kv[p, t] = 8.0 - (4 * t + p // 32 + 1)
    kv[0:32, 4] = -1.0     # tail rows 0-31: hat0 ramp = -x8 - 7
    kv[32:64, 4] = 1.0     # tail rows 32-63: hat16 ramp = x8 - 7

    rep = np.zeros((32, 128), dtype=bf)
    for p in range(128):
        rep[p % 32, p] = 1.0
    ones = np.ones((1, SPAT_PAD), dtype=bf)

    x8 = (np.asarray(x, dtype=np.float32) * 8.0).astype(bf)
    in_maps = []
    for core in range(N_CORES):
        b, half = divmod(core, 2)
        r0 = half * ROWS_PER_CORE
        xs = np.zeros((32, SPAT_PAD + 128), dtype=bf)
        xs[:, :SPAT] = x8[b, r0:r0 + IN_ROWS].reshape(SPAT, C).T
        xs[:, SPAT_PAD:] = rep
        in_maps.append({"x8t": xs, "w": w_host, "kv": kv, "ones": ones})
    return in_maps


def kernel(x, control_points, w_spline, w_silu, bias):
    in_maps = _make_in_maps(x, control_points, w_spline, w_silu, bias)
    nc = _get_program()
    res = run_bass_kernel_spmd(nc, in_maps, list(range(N_CORES)))

    out = np.empty((B, HO, WO, F), dtype=np.float32)
    for core in range(N_CORES):
        b, half = divmod(core, 2)
        r0 = half * ROWS_PER_CORE
        y2 = res.results[core]["y"].astype(np.float32)   # [128, 2048] fp16
        y = y2[0:F] + y2[F:128]                          # [64, 2048]
        out[b, r0:r0 + ROWS_PER_CORE] = (
            y.reshape(F, ROWS_PER_CORE, WO).transpose(1, 2, 0))
    return out


# revision 27
# speedup vs baseline: 1.0624x; 1.0142x over previous
"""KANvolution Trainium2 Bass kernel (v4: hat basis, bf16, col-tiled PE,
PE-side input replication, two-phase chunk pipeline, fp16 output).

Math: per patch element x and per (f,c,ki,kj):
    K(x) = w_spline * sum_g hat_g(clip(x)) * cp_g  +  w_silu * silu(x)
with hat_g the normalized linear B-spline (tent) basis on the 17-knot
grid in [-1,1] (hat sum == 1, so the reference's /(sum+1e-8) is a
constant 1/(1+1e-8) scale folded into the weights).

The tent basis is evaluated directly:
    nhat_g(x) = min(|8x - k_g| - 1, 0)      ( = -hat_g,  k_g = 8*grid_g )
with spline weights negated host-side.  Clipping x to [-1,1] only
matters for the two edge knots (interior tents vanish outside the grid
anyway), so knots 1..15 are computed straight from x and the edge knots
live in the off-critical-path tail k-tile.

k-tiles per tap (ki,kj):
    t=0..3 : knots 4t+1 .. 4t+4  x 32 ch   (slot (3,3) is a zero-weight pad)
    t=4    : hat0(32) + hat16(32) + silu(32) + bias-ones(1) = 97 rows
9 taps x 5 k-tiles x 4 row-chunks = 180 matmuls [K<=128, N=64] x [K, 512].

PE usage: F=64 fills half the 128-wide array, so matmuls alternate
between array column groups (tile_position via PSUM slice base
partition); the groups stream concurrently through separate XBUSes.
Each (t,tap) weight is loaded once per chunk-pair (LDWEIGHTS hides
under the other group's matmuls).  Chunks {0,1} complete in phase 1 so
their output DMA overlaps phase 2's matmuls.

Input x is DMAed once ([32, 2304] bf16) and replicated to 128
partitions by a K=32 matmul against a replication matrix (this also
warms the PE clock).  Outputs leave as fp16 per-column-group halves;
the host sums them.

Sharding: 8 cores = (batch b, output-row half); each core computes
(32, 64, 64) of the output.
"""

import numpy as np
from contextlib import ExitStack

import concourse.bacc as bacc
import concourse.mybir as mybir
import concourse.tile as tile
from concourse.bass_utils import run_bass_kernel_spmd

# Problem constants (hardcoded per harness contract)
B, H, W, C, F = 4, 66, 66, 32, 64
KH = KW = 3
G = 16                                   # spline intervals; G+1 = 17 knots
HO, WO = H - KH + 1, W - KW + 1          # 64, 64
N_CORES = 8
ROWS_PER_CORE = HO // 2                  # 32 output rows
IN_ROWS = ROWS_PER_CORE + KH - 1         # 34 input rows
SPAT = IN_ROWS * W                       # 2244 input spatial positions
SPAT_PAD = 2304                          # pad to 18*128
N_TAPS = KH * KW                         # 9
N_KTILES = 5
K_TAIL = 97                              # hat0(32)+hat16(32)+silu(32)+bias(1)
CHUNK_ROWS = 8                           # output rows per matmul chunk
N_CHUNKS = ROWS_PER_CORE // CHUNK_ROWS   # 4
NFREE = CHUNK_ROWS * WO                  # 512 moving-dim per matmul
SLABS = [(0, 576), (576, 1152), (1152, 1728), (1728, 2304)]  # feature slabs
N_WARMUP = 8                             # HAM warm-up matmuls (K=128)

_COMPILED = None  # cached (nc) program


def _build_weights(control_points, w_spline, w_silu, bias):
    """[128, 45*64] bf16; column block (t*9+tap)*64 holds k-tile t of tap.

    t<4 : row r*32+c = knot 4t+r+1 (negated); slot (t,r)==(3,3) is zero.
    t=4 : rows 0-31 knot 0 (pos), 32-63 knot 16 (pos), 64-95 w_silu,
          row 96 bias (tap 0 only).
    """
    import ml_dtypes
    cp = control_points.astype(np.float64)
    ws = w_spline.astype(np.float64)
    v = ws[..., None] * cp / (1.0 + 1e-8)          # (F, C, 3, 3, 17)

    w_all = np.zeros((N_KTILES, N_TAPS, 128, F), dtype=np.float64)
    for i in range(KH):
        for j in range(KW):
            tap = i * KW + j
            for t in range(4):
                for r in range(4):
                    if t == 3 and r == 3:
                        continue                   # zero-weight pad slot
                    g = 4 * t + r + 1
                    w_all[t, tap, r * 32:(r + 1) * 32, :] = -v[:, :, i, j, g].T
            w_all[4, tap, 0:32, :] = v[:, :, i, j, 0].T
            w_all[4, tap, 32:64, :] = v[:, :, i, j, 16].T
            w_all[4, tap, 64:96, :] = w_silu[:, :, i, j].astype(np.float64).T
    w_all[4, 0, 96, :] = bias.astype(np.float64)
    w_host = w_all.transpose(2, 0, 1, 3).reshape(128, N_KTILES * N_TAPS * F)
    return np.ascontiguousarray(w_host.astype(ml_dtypes.bfloat16))


def _build_program():
    nc = bacc.Bacc("TRN2", target_bir_lowering=False, debug=False,
                   num_devices=N_CORES)
    f32 = mybir.dt.float32
    bf16 = mybir.dt.bfloat16
    fp16 = mybir.dt.float16
    AF = mybir.ActivationFunctionType
    OP = mybir.AluOpType
    import os
    # CoreSim has no Silu; swap in Sigmoid for sim-only structure checks.
    AF_SILU = AF.Sigmoid if os.environ.get("KAN_SIM_SAFE") else AF.Silu

    x_in = nc.declare_dram_parameter("x8t", [32, SPAT_PAD + 128], bf16,
                                    isOutput=False)
    w_in = nc.declare_dram_parameter("w", [128, N_KTILES * N_TAPS * F], bf16,
                                     isOutput=False)
    kv_in = nc.declare_dram_parameter("kv", [128, 8], f32, isOutput=False)
    ones_in = nc.declare_dram_parameter("ones", [1, SPAT_PAD], bf16,
                                        isOutput=False)
    y_out = nc.declare_dram_parameter("y", [128, N_CHUNKS * NFREE], fp16,
                                      isOutput=True)

    with tile.TileContext(nc) as tc:
        with ExitStack() as ctx:
            sb = ctx.enter_context(tc.tile_pool(name="sb", bufs=1))
            ps = ctx.enter_context(tc.tile_pool(name="ps", bufs=1, space="PSUM"))
            ob = ctx.enter_context(tc.tile_pool(name="ob", bufs=1))

            # --- input DMAs: all three DMA-capable queues, need-order;
            # the replication matrix rides in x8t's last 128 columns ---
            kv_sb = sb.tile([128, 8], f32, tag="kv")
            nc.sync.dma_start(kv_sb[:], kv_in[:])
            x_sb = sb.tile([32, SPAT_PAD + 128], bf16, tag="xsb")
            nc.sync.dma_start(x_sb[0:11, :], x_in[0:11, :])
            nc.scalar.dma_start(x_sb[11:22, :], x_in[11:22, :])
            nc.gpsimd.dma_start(x_sb[22:32, :], x_in[22:32, :])
            rep_sb = x_sb[:, SPAT_PAD:SPAT_PAD + 128]
            w_sb = sb.tile([128, N_KTILES * N_TAPS * F], bf16, tag="w")
            nc.sync.dma_start(w_sb[0:44, 0:576], w_in[0:44, 0:576])    # t=0
            nc.scalar.dma_start(w_sb[44:88, 0:576], w_in[44:88, 0:576])
            nc.gpsimd.dma_start(w_sb[88:128, 0:576], w_in[88:128, 0:576])
            nc.gpsimd.dma_start(w_sb[:, 576:1152], w_in[:, 576:1152])  # t=1
            nc.sync.dma_start(w_sb[:, 1152:1728], w_in[:, 1152:1728])
            nc.scalar.dma_start(w_sb[:, 1728:2304], w_in[:, 1728:2304])
            nc.gpsimd.dma_start(w_sb[:, 2304:2880], w_in[:, 2304:2880])

            # feature tiles
            x_rep = sb.tile([128, SPAT_PAD], bf16, tag="xrep")
            u64 = sb.tile([64, SPAT_PAD], bf16, tag="u64")
            tb = [sb.tile([128, SPAT_PAD], bf16, name=f"tb{u}", tag=f"tb{u}")
                  for u in range(2)]
            nhat = [sb.tile([128, SPAT_PAD], bf16, name=f"nh{t}", tag=f"nh{t}")
                    for t in range(N_KTILES)]
            nc.sync.dma_start(nhat[4][96:97, :], ones_in[:])   # bias row = 1.0

            # PSUM: 4 chunk tiles x 2 banks (col-group A in bank 0 /
            # partitions 0-63, group B in bank 1 / partitions 64-127 — one
            # accumulation group per zero region).
            P = [ps.tile([128, 2 * NFREE], f32, name=f"po{q}", tag=f"po{q}")
                 for q in range(N_CHUNKS)]

            # HAM warm-up: K=128 junk matmuls on a zero-filled tile (K=32
            # matmuls don't generate enough array activity to unthrottle
            # the clock).  They write P[3]'s B region; the real
            # accumulation's start=True clears it.  Phase A runs before the
            # input-dependent work, phase B bridges until features land.
            nc.vector.memset(tb[0][:, 0:NFREE], 0.0)
            for u in range(N_WARMUP):
                nc.tensor.matmul(P[3][64:128, NFREE:2 * NFREE],
                                 tb[0][:, 0:F], tb[0][:, 0:NFREE],
                                 start=True, stop=True)

            # --- replicate x to 128 partitions via K=32 matmuls against
            # the replication matrix ---
            for u in range(5):
                c0 = 512 * u
                c1 = min(c0 + 512, SPAT_PAD)
                pr = P[u % 4][:, (u // 4) * NFREE:(u // 4) * NFREE + c1 - c0]
                nc.tensor.matmul(pr, rep_sb, x_sb[:, c0:c1],
                                 start=True, stop=True)
                hm = (c1 - c0) // 2
                nc.scalar.copy(x_rep[:, c0:c0 + hm], pr[:, 0:hm])
                nc.vector.tensor_copy(x_rep[:, c0 + hm:c1], pr[:, hm:])
            for u in range(6):      # phase-B warm-up bridge
                nc.tensor.matmul(P[3][64:128, NFREE:2 * NFREE],
                                 tb[0][:, 0:F], tb[0][:, 0:NFREE],
                                 start=True, stop=True)

            def features(t):
                """All three column slabs of k-tile t (t-major emission)."""
                for a, b in SLABS:
                    cs = slice(a, b)
                    if t < 4:
                        tbt = tb[t % 2]
                        # |x8 - k| on ACT (per-partition bias = -k); interior
                        # knots need no clipping.
                        nc.scalar.activation(tbt[:, cs], x_rep[:, cs], AF.Abs,
                                             bias=kv_sb[:, t:t + 1], scale=1.0)
                        nc.vector.tensor_scalar(nhat[t][:, cs], tbt[:, cs],
                                                1.0, 0.0, OP.subtract, OP.min)
                    else:
                        # tail: hat0 = clamp(-x8-7, 0, 1) (rows 0-31),
                        # hat16 = clamp(x8-7, 0, 1) (rows 32-63) — positive
                        # tents, weights not negated; silu from raw x.
                        nc.vector.tensor_scalar(u64[:, cs], x_rep[0:64, cs],
                                                kv_sb[0:64, 4:5], -7.0,
                                                OP.mult, OP.add)
                        nc.vector.tensor_scalar(nhat[4][0:64, cs], u64[:, cs],
                                                0.0, 1.0, OP.max, OP.min)
                        nc.scalar.activation(nhat[4][64:96, cs],
                                             x_rep[32:64, cs],
                                             AF_SILU, scale=0.125)

            order = [(t, tap) for t in range(N_KTILES) for tap in range(N_TAPS)]
            started = set()

            def emit_mm(n, t, tap, qlist, stop):
                g_ = n % 2
                i, j = divmod(tap, KW)
                kk = 128 if t < 4 else K_TAIL
                col = (t * N_TAPS + tap) * F
                lhsT = w_sb[0:kk, col:col + F]
                for q in qlist:
                    base = (CHUNK_ROWS * q + i) * W
                    rhs = (nhat[t][0:kk, base:base + CHUNK_ROWS * W]
                           .rearrange("p (r w) -> p r w", w=W)
                           [:, :, j:j + WO])
                    nc.tensor.matmul(
                        P[q][F * g_:F * (g_ + 1),
                             NFREE * g_:NFREE * (g_ + 1)]
                            .rearrange("f (r w) -> f r w", w=WO),
                        lhsT, rhs,
                        start=((q, g_) not in started), stop=stop,
                    )
                    started.add((q, g_))

            def emit_out(q):
                # PSUM -> SBUF as fp16 halves (host sums them); ACT and DVE
                # copy a half each, each half DMAs from its own queue.
                stage = ob.tile([128, NFREE], fp16, tag=f"stage{q}")
                hm = NFREE // 2
                nc.scalar.copy(stage[0:F, 0:hm], P[q][0:F, 0:hm])
                nc.vector.tensor_copy(stage[0:F, hm:NFREE],
                                      P[q][0:F, hm:NFREE])
                nc.sync.dma_start(y_out[0:F, NFREE * q:NFREE * (q + 1)],
                                  stage[0:F, :])
                nc.vector.tensor_copy(stage[F:128, 0:hm],
                                      P[q][F:128, NFREE:NFREE + hm])
                nc.scalar.copy(stage[F:128, hm:NFREE],
                               P[q][F:128, NFREE + hm:2 * NFREE])
                nc.gpsimd.dma_start(y_out[F:128, NFREE * q:NFREE * (q + 1)],
                                    stage[F:128, :])

            # hat tiles t0-t3 for all 4 chunks (weights stay loaded across
            # the 4 matmuls); the tail tile closes chunks {0,1} first so
            # their output DMA overlaps the rest.
            for n, (t, tap) in enumerate(order):
                if tap == 0:
                    features(t)
                if t < 4:
                    emit_mm(n, t, tap, (0, 1, 2, 3), stop=False)
            for n, (t, tap) in enumerate(order):
                if t == 4:
                    emit_mm(n, t, tap, (0, 1), stop=(n >= len(order) - 2))
            emit_out(0)
            emit_out(1)
            for n, (t, tap) in enumerate(order):
                if t == 4:
                    emit_mm(n, t, tap, (2, 3), stop=(n >= len(order) - 2))
            emit_out(2)
            emit_out(3)

    nc.compile()
    return nc


def _get_program():
    global _COMPILED
    if _COMPILED is None:
        _COMPILED = _build_program()
    return _COMPILED


def _make_in_maps(x, control_points, w_spline, w_silu, bias):
    import ml_dtypes
    bf = ml_dtypes.bfloat16
    w_host = _build_weights(control_points, w_spline, w_silu, bias)

    # ACT Abs bias constants: tb = Abs(x8 + kv); interior knot index
    # n = 4t + r + 1 sits at k = n - 8, so kv = 8 - n.
    kv = np.zeros((128, 8), dtype=np.float32)
    for t in range(4):
        for p in range(128):
            kv[p, t] = 8.0 - (4 * t + p // 32 + 1)
    kv[0:32, 4] = -1.0     # tail rows 0-31: hat0 ramp = -x8 - 7
    kv[32:64, 4] = 1.0     # tail rows 32-63: hat16 ramp = x8 - 7

    rep = np.zeros((32, 128), dtype=bf)
    for p in range(128):
        rep[p % 32, p] = 1.0
    ones = np.ones((1, SPAT_PAD), dtype=bf)

    x8 = (np.asarray(x, dtype=np.float32) * 8.0).astype(bf)
    in_maps = []
    for core in range(N_CORES):
        b, half = divmod(core, 2)
        r0 = half * ROWS_PER_CORE
        xs = np.zeros((32, SPAT_PAD + 128), dtype=bf)
        xs[:, :SPAT] = x8[b, r0:r0 + IN_ROWS].reshape(SPAT, C).T
        xs[:, SPAT_PAD:] = rep
        in_maps.append({"x8t": xs, "w": w_host, "kv": kv, "ones": ones})
    return in_maps


def kernel(x, control_points, w_spline, w_silu, bias):
    in_maps = _make_in_maps(x, control_points, w_spline, w_silu, bias)
    nc = _get_program()
    res = run_bass_kernel_spmd(nc, in_maps, list(range(N_CORES)))

    out = np.empty((B, HO, WO, F), dtype=np.float32)
    for core in range(N_CORES):
        b, half = divmod(core, 2)
        r0 = half * ROWS_PER_CORE
        y2 = res.results[core]["y"].astype(np.float32)   # [128, 2048] fp16
        y = y2[0:F] + y2[F:128]                          # [64, 2048]
        out[b, r0:r0 + ROWS_PER_CORE] = (
            y.reshape(F, ROWS_PER_CORE, WO).transpose(1, 2, 0))
    return out
